# revision 1
# baseline (speedup 1.0000x reference)
"""Trainium2 Bass kernel for ConcatenateSphericalSignals.

The op: concat(signal1, signal2) along the channel dim, then apply a
768x768 one-hot permutation matrix to the channel dim (einsum
'dc,ncba->ndba').  Since the mixing matrix is a permutation, the whole
thing is a channel-gather, and because the permutation merge-sorts
contiguous blocks, it collapses to a handful of large contiguous block
copies per sample.  We shard the batch dim N=16 across 8 cores (2
samples/core) and issue one strided DRAM->DRAM DMA per block.
"""

import numpy as np

import concourse.bass as bass
import concourse.mybir as mybir
from concourse.bass_utils import run_bass_kernel_spmd

# Problem shape (hardcoded per harness contract).
N, F1, F2 = 16, 288, 480
FO = F1 + F2
B, A = 64, 64
BA = B * A
NCORES = 8
NLOC = N // NCORES  # samples per core

# Test harness hooks: set TRACE=True before calling kernel() to collect a
# profile; LAST_RESULT then holds the BassKernelResults.
TRACE = False
LAST_RESULT = None

_module_cache: dict = {}


def _copy_plan(mixing_matrix: np.ndarray):
    """Decompose the one-hot permutation matrix into maximal contiguous
    block copies: (src_tensor_idx, src_chan_start, dst_chan_start, length)."""
    M = np.asarray(mixing_matrix)
    assert M.shape == (FO, FO)
    perm = M.argmax(axis=1).astype(np.int64)
    # Verify it really is a permutation one-hot matrix (cheap host check);
    # the kernel below is only valid under that assumption.
    assert np.array_equal(np.sort(perm), np.arange(FO)), "not a permutation"
    ref = np.zeros_like(M)
    ref[np.arange(FO), perm] = 1.0
    assert np.array_equal(ref, M), "mixing matrix is not one-hot"

    runs = []
    d = 0
    while d < FO:
        c0 = int(perm[d])
        L = 1
        while (
            d + L < FO
            and int(perm[d + L]) == c0 + L
            and (c0 < F1) == (c0 + L < F1)  # stay within one source tensor
        ):
            L += 1
        if c0 < F1:
            runs.append((0, c0, d, L))
        else:
            runs.append((1, c0 - F1, d, L))
        d += L
    return tuple(runs)


def _build_module(runs):
    nc = bass.Bass()
    s1 = nc.declare_dram_parameter(
        "signal1", [NLOC, F1, BA], mybir.dt.float32, isOutput=False
    )
    s2 = nc.declare_dram_parameter(
        "signal2", [NLOC, F2, BA], mybir.dt.float32, isOutput=False
    )
    out = nc.declare_dram_parameter(
        "out", [NLOC, FO, BA], mybir.dt.float32, isOutput=True
    )
    srcs = [s1, s2]
    with nc.Block() as block, nc.semaphore("dma_sem") as dma_sem:

        @block.sync
        def _(sync):
            for which, c0, d0, L in runs:
                sync.dma_start(
                    out=out[:, d0 : d0 + L, :],
                    in_=srcs[which][:, c0 : c0 + L, :],
                ).then_inc(dma_sem, 16)
            sync.wait_ge(dma_sem, 16 * len(runs))

    return nc


def kernel(signal1: np.ndarray, signal2: np.ndarray, mixing_matrix: np.ndarray):
    global LAST_RESULT
    signal1 = np.ascontiguousarray(np.asarray(signal1, dtype=np.float32))
    signal2 = np.ascontiguousarray(np.asarray(signal2, dtype=np.float32))
    assert signal1.shape == (N, F1, B, A)
    assert signal2.shape == (N, F2, B, A)

    runs = _copy_plan(mixing_matrix)
    nc = _module_cache.get(runs)
    if nc is None:
        nc = _build_module(runs)
        _module_cache[runs] = nc

    s1 = signal1.reshape(N, F1, BA)
    s2 = signal2.reshape(N, F2, BA)
    core_ids = list(range(NCORES))
    in_maps = [
        {
            "signal1": s1[c * NLOC : (c + 1) * NLOC],
            "signal2": s2[c * NLOC : (c + 1) * NLOC],
        }
        for c in core_ids
    ]

    res = run_bass_kernel_spmd(nc, in_maps, core_ids, trace=TRACE)
    LAST_RESULT = res

    out = np.concatenate([r["out"] for r in res.results], axis=0)
    return out.reshape(N, FO, B, A)


# revision 2
# speedup vs baseline: 1.7813x; 1.7813x over previous
"""Trainium2 Bass kernel for ConcatenateSphericalSignals.

The op: concat(signal1, signal2) along the channel dim, then apply a
768x768 one-hot permutation matrix to the channel dim (einsum
'dc,ncba->ndba').  Since the mixing matrix is a permutation, the whole
thing is a channel-gather, and because the permutation merge-sorts
contiguous blocks, it collapses to a handful of large contiguous block
copies per sample.  We shard the batch dim N=16 across 8 cores (2
samples/core) and issue one strided DRAM->DRAM DMA per block.
"""

import numpy as np

import concourse.bass as bass
import concourse.mybir as mybir
from concourse.bass_utils import run_bass_kernel_spmd

# Problem shape (hardcoded per harness contract).
N, F1, F2 = 16, 288, 480
FO = F1 + F2
B, A = 64, 64
BA = B * A
NCORES = 8
NLOC = N // NCORES  # samples per core

# Test harness hooks: set TRACE=True before calling kernel() to collect a
# profile; LAST_RESULT then holds the BassKernelResults.
TRACE = False
LAST_RESULT = None

_module_cache: dict = {}


def _copy_plan(mixing_matrix: np.ndarray):
    """Decompose the one-hot permutation matrix into maximal contiguous
    block copies: (src_tensor_idx, src_chan_start, dst_chan_start, length)."""
    M = np.asarray(mixing_matrix)
    assert M.shape == (FO, FO)
    perm = M.argmax(axis=1).astype(np.int64)
    # Verify it really is a permutation one-hot matrix (cheap host check);
    # the kernel below is only valid under that assumption.
    assert np.array_equal(np.sort(perm), np.arange(FO)), "not a permutation"
    ref = np.zeros_like(M)
    ref[np.arange(FO), perm] = 1.0
    assert np.array_equal(ref, M), "mixing matrix is not one-hot"

    runs = []
    d = 0
    while d < FO:
        c0 = int(perm[d])
        L = 1
        while (
            d + L < FO
            and int(perm[d + L]) == c0 + L
            and (c0 < F1) == (c0 + L < F1)  # stay within one source tensor
        ):
            L += 1
        if c0 < F1:
            runs.append((0, c0, d, L))
        else:
            runs.append((1, c0 - F1, d, L))
        d += L
    return tuple(runs)


def _build_module(runs):
    nc = bass.Bass()
    s1 = nc.declare_dram_parameter(
        "signal1", [NLOC, F1, BA], mybir.dt.float32, isOutput=False
    )
    s2 = nc.declare_dram_parameter(
        "signal2", [NLOC, F2, BA], mybir.dt.float32, isOutput=False
    )
    out = nc.declare_dram_parameter(
        "out", [NLOC, FO, BA], mybir.dt.float32, isOutput=True
    )
    srcs = [s1, s2]
    with nc.Block() as block, nc.semaphore("dma_sem") as dma_sem:

        @block.gpsimd
        def _(gpsimd):
            for which, c0, d0, L in runs:
                gpsimd.dma_start(
                    out=out[:, d0 : d0 + L, :],
                    in_=srcs[which][:, c0 : c0 + L, :],
                ).then_inc(dma_sem, 16)
            gpsimd.wait_ge(dma_sem, 16 * len(runs))

    return nc


def kernel(signal1: np.ndarray, signal2: np.ndarray, mixing_matrix: np.ndarray):
    global LAST_RESULT
    signal1 = np.ascontiguousarray(np.asarray(signal1, dtype=np.float32))
    signal2 = np.ascontiguousarray(np.asarray(signal2, dtype=np.float32))
    assert signal1.shape == (N, F1, B, A)
    assert signal2.shape == (N, F2, B, A)

    runs = _copy_plan(mixing_matrix)
    nc = _module_cache.get(runs)
    if nc is None:
        nc = _build_module(runs)
        _module_cache[runs] = nc

    s1 = signal1.reshape(N, F1, BA)
    s2 = signal2.reshape(N, F2, BA)
    core_ids = list(range(NCORES))
    in_maps = [
        {
            "signal1": s1[c * NLOC : (c + 1) * NLOC],
            "signal2": s2[c * NLOC : (c + 1) * NLOC],
        }
        for c in core_ids
    ]

    res = run_bass_kernel_spmd(nc, in_maps, core_ids, trace=TRACE)
    LAST_RESULT = res

    out = np.concatenate([r["out"] for r in res.results], axis=0)
    return out.reshape(N, FO, B, A)


# revision 3
# speedup vs baseline: 1.9061x; 1.0701x over previous
"""Trainium2 Bass kernel for ConcatenateSphericalSignals.

The op: concat(signal1, signal2) along the channel dim, then apply a
768x768 one-hot permutation matrix to the channel dim (einsum
'dc,ncba->ndba').  Since the mixing matrix is a permutation, the whole
thing is a channel-gather, and because the permutation merge-sorts
contiguous blocks, it collapses to a handful of large contiguous block
copies per sample.  We shard the batch dim N=16 across 8 cores (2
samples/core) and issue one strided DRAM->DRAM DMA per block.
"""

import numpy as np

import concourse.bass as bass
import concourse.mybir as mybir
from concourse.bass_utils import run_bass_kernel_spmd

# Problem shape (hardcoded per harness contract).
N, F1, F2 = 16, 288, 480
FO = F1 + F2
B, A = 64, 64
BA = B * A
NCORES = 8
NLOC = N // NCORES  # samples per core

# Test harness hooks: set TRACE=True before calling kernel() to collect a
# profile; LAST_RESULT then holds the BassKernelResults.
TRACE = False
LAST_RESULT = None

_module_cache: dict = {}


def _copy_plan(mixing_matrix: np.ndarray):
    """Decompose the one-hot permutation matrix into maximal contiguous
    block copies: (src_tensor_idx, src_chan_start, dst_chan_start, length)."""
    M = np.asarray(mixing_matrix)
    assert M.shape == (FO, FO)
    perm = M.argmax(axis=1).astype(np.int64)
    # Verify it really is a permutation one-hot matrix (cheap host check);
    # the kernel below is only valid under that assumption.
    assert np.array_equal(np.sort(perm), np.arange(FO)), "not a permutation"
    ref = np.zeros_like(M)
    ref[np.arange(FO), perm] = 1.0
    assert np.array_equal(ref, M), "mixing matrix is not one-hot"

    runs = []
    d = 0
    while d < FO:
        c0 = int(perm[d])
        L = 1
        while (
            d + L < FO
            and int(perm[d + L]) == c0 + L
            and (c0 < F1) == (c0 + L < F1)  # stay within one source tensor
        ):
            L += 1
        if c0 < F1:
            runs.append((0, c0, d, L))
        else:
            runs.append((1, c0 - F1, d, L))
        d += L
    return tuple(runs)


def _build_module(runs):
    nc = bass.Bass()
    s1 = nc.declare_dram_parameter(
        "signal1", [NLOC, F1, BA], mybir.dt.float32, isOutput=False
    )
    s2 = nc.declare_dram_parameter(
        "signal2", [NLOC, F2, BA], mybir.dt.float32, isOutput=False
    )
    out = nc.declare_dram_parameter(
        "out", [NLOC, FO, BA], mybir.dt.float32, isOutput=True
    )
    srcs = [s1, s2]
    with nc.Block() as block, nc.semaphore("dma_sem") as dma_sem:

        @block.gpsimd
        def _(gpsimd):
            # SWDGE assigns one descriptor chain per outer-AP element to one
            # SDMA engine; reshape each block copy to 16 chunks per sample so
            # all 16 engines get even work.
            for which, c0, d0, L in runs:
                k = 16 if L % 16 == 0 else 1
                gpsimd.dma_start(
                    out=out[:, d0 : d0 + L, :].rearrange(
                        "n (k c) f -> n k (c f)", k=k
                    ),
                    in_=srcs[which][:, c0 : c0 + L, :].rearrange(
                        "n (k c) f -> n k (c f)", k=k
                    ),
                ).then_inc(dma_sem, 16)
            gpsimd.wait_ge(dma_sem, 16 * len(runs))

    return nc


def kernel(signal1: np.ndarray, signal2: np.ndarray, mixing_matrix: np.ndarray):
    global LAST_RESULT
    signal1 = np.ascontiguousarray(np.asarray(signal1, dtype=np.float32))
    signal2 = np.ascontiguousarray(np.asarray(signal2, dtype=np.float32))
    assert signal1.shape == (N, F1, B, A)
    assert signal2.shape == (N, F2, B, A)

    runs = _copy_plan(mixing_matrix)
    nc = _module_cache.get(runs)
    if nc is None:
        nc = _build_module(runs)
        _module_cache[runs] = nc

    s1 = signal1.reshape(N, F1, BA)
    s2 = signal2.reshape(N, F2, BA)
    core_ids = list(range(NCORES))
    in_maps = [
        {
            "signal1": s1[c * NLOC : (c + 1) * NLOC],
            "signal2": s2[c * NLOC : (c + 1) * NLOC],
        }
        for c in core_ids
    ]

    res = run_bass_kernel_spmd(nc, in_maps, core_ids, trace=TRACE)
    LAST_RESULT = res

    out = np.concatenate([r["out"] for r in res.results], axis=0)
    return out.reshape(N, FO, B, A)


# revision 4
# speedup vs baseline: 5.3132x; 2.7875x over previous
"""Trainium2 Bass kernel for ConcatenateSphericalSignals.

The op: concat(signal1, signal2) along the channel dim, then apply a
768x768 one-hot permutation matrix to the channel dim (einsum
'dc,ncba->ndba').  Since the mixing matrix is a permutation, the whole
thing is a channel-gather, and because the permutation merge-sorts
contiguous blocks, it collapses to a handful of large contiguous block
copies per sample.  We shard the batch dim N=16 across 8 cores (2
samples/core) and issue one strided DRAM->DRAM DMA per block.
"""

import numpy as np

import concourse.bass as bass
import concourse.mybir as mybir
from concourse.bass_utils import run_bass_kernel_spmd

# Problem shape (hardcoded per harness contract).
N, F1, F2 = 16, 288, 480
FO = F1 + F2
B, A = 64, 64
BA = B * A
NCORES = 8
NLOC = N // NCORES  # samples per core

# Test harness hooks: set TRACE=True before calling kernel() to collect a
# profile; LAST_RESULT then holds the BassKernelResults.
TRACE = False
LAST_RESULT = None

_module_cache: dict = {}


def _copy_plan(mixing_matrix: np.ndarray):
    """Decompose the one-hot permutation matrix into maximal contiguous
    block copies: (src_tensor_idx, src_chan_start, dst_chan_start, length)."""
    M = np.asarray(mixing_matrix)
    assert M.shape == (FO, FO)
    perm = M.argmax(axis=1).astype(np.int64)
    # Verify it really is a permutation one-hot matrix (cheap host check);
    # the kernel below is only valid under that assumption.
    assert np.array_equal(np.sort(perm), np.arange(FO)), "not a permutation"
    ref = np.zeros_like(M)
    ref[np.arange(FO), perm] = 1.0
    assert np.array_equal(ref, M), "mixing matrix is not one-hot"

    runs = []
    d = 0
    while d < FO:
        c0 = int(perm[d])
        L = 1
        while (
            d + L < FO
            and int(perm[d + L]) == c0 + L
            and (c0 < F1) == (c0 + L < F1)  # stay within one source tensor
        ):
            L += 1
        if c0 < F1:
            runs.append((0, c0, d, L))
        else:
            runs.append((1, c0 - F1, d, L))
        d += L
    return tuple(runs)


def _build_module(runs):
    nc = bass.Bass()
    s1 = nc.declare_dram_parameter(
        "signal1", [NLOC, F1, BA], mybir.dt.float32, isOutput=False
    )
    s2 = nc.declare_dram_parameter(
        "signal2", [NLOC, F2, BA], mybir.dt.float32, isOutput=False
    )
    out = nc.declare_dram_parameter(
        "out", [NLOC, FO, BA], mybir.dt.float32, isOutput=True
    )
    srcs = [s1, s2]
    with nc.Block() as block, nc.semaphore("dma_sem") as dma_sem:

        @block.gpsimd
        def _(gpsimd):
            # DMA descriptor chains are sprayed across the 16 SDMA engines
            # over the outermost AP dim.  A flat 1D AP takes the
            # "single-dim" path in balance_dma_aps, which splits it into
            # 64KiB rows with a 16-multiple row count — even engine spread.
            ndma = 0
            for which, c0, d0, L in runs:
                for n in range(NLOC):
                    gpsimd.dma_start(
                        out=out[n, d0 : d0 + L, :].rearrange("c f -> (c f)"),
                        in_=srcs[which][n, c0 : c0 + L, :].rearrange(
                            "c f -> (c f)"
                        ),
                    ).then_inc(dma_sem, 16)
                    ndma += 1
            gpsimd.wait_ge(dma_sem, 16 * ndma)

    return nc


def kernel(signal1: np.ndarray, signal2: np.ndarray, mixing_matrix: np.ndarray):
    global LAST_RESULT
    signal1 = np.ascontiguousarray(np.asarray(signal1, dtype=np.float32))
    signal2 = np.ascontiguousarray(np.asarray(signal2, dtype=np.float32))
    assert signal1.shape == (N, F1, B, A)
    assert signal2.shape == (N, F2, B, A)

    runs = _copy_plan(mixing_matrix)
    nc = _module_cache.get(runs)
    if nc is None:
        nc = _build_module(runs)
        _module_cache[runs] = nc

    s1 = signal1.reshape(N, F1, BA)
    s2 = signal2.reshape(N, F2, BA)
    core_ids = list(range(NCORES))
    in_maps = [
        {
            "signal1": s1[c * NLOC : (c + 1) * NLOC],
            "signal2": s2[c * NLOC : (c + 1) * NLOC],
        }
        for c in core_ids
    ]

    res = run_bass_kernel_spmd(nc, in_maps, core_ids, trace=TRACE)
    LAST_RESULT = res

    out = np.concatenate([r["out"] for r in res.results], axis=0)
    return out.reshape(N, FO, B, A)
